# revision 2
# baseline (speedup 1.0000x reference)
"""Diagonal-Gaussian KL loss on 8 Trainium2 NeuronCores — v2.

KL(p || q) summed over batch:
  0.5 * [ sum(sigma_q - sigma_p) + sum(exp(sigma_p - sigma_q))
          + sum((mu_q-mu_p)^2 * exp(-sigma_q)) - B*D ]

Data-parallel over batch: each core handles a [1024, 2048] shard of the four
inputs, viewed as 8 row-tiles of [128, 2048].

All inputs are fed as bf16 (rel tolerance is 2e-2; measured end error ~4e-5).
HBM traffic: 8 B/elem = 16.8 MB/core ~ 41 us at the measured ~406 GB/s
streaming rate. All 8 tiles are SBUF-resident, so input DMA free-runs.

GPSIMD is deliberately unused: it shares an SBUF port with the DVE, and any
concurrent GPSIMD tensor op slows DVE 2x-mode tensor_tensor by ~4x (measured
1.22us -> 4.85us).

Per tile i (DVE does 4 bf16 TT passes, all in 2x mode, 4.9us/tile total,
under the 5.2us/tile DMA pace; ACT ~3.9us/tile; PE ~3.4us/tile):
  DVE : a = sigma_p - sigma_q          e3/u/m: 1.22us each
        d = mu_q - mu_p
        u = d * e3;  m = u * u
  ACT : e3 = exp(-0.5*sigma_q)         (2.0us)
        exp(a) accum-> acc_e           (batched over 2 tiles for i<6;
                                        per-tile for i>=6 to shorten the tail)
  PE  : psum_a += ones^T @ a chunks    (4 x 512-col bf16 matmuls, ~1.7us)
        psum_m += ones^T @ m chunks
Tail: ACT copies psum_a|psum_m -> SBUF, DMA out acc_e [128,5] + sums [1,1024].
Host: kl = 0.5 * (-sum(psum_a) + sum(acc_e) + sum(psum_m) - B*D), f64.
"""

from contextlib import ExitStack

import numpy as np
import ml_dtypes

import concourse.bass as bass
from concourse import mybir
from concourse.bass_utils import run_bass_kernel_spmd

B, D = 8192, 2048
NCORES = 8
ROWS = B // NCORES
P = 128
NT = ROWS // P  # 8 tiles per core

F32 = mybir.dt.float32
BF16 = mybir.dt.bfloat16
NPBF16 = ml_dtypes.bfloat16
Exp = mybir.ActivationFunctionType.Exp
Alu = mybir.AluOpType


def _build_nc():
    nc = bass.Bass(trn_type="TRN2", target_bir_lowering=False)

    xs = nc.dram_tensor("xs", [ROWS, 2 * D], BF16, kind="ExternalInput")
    xm = nc.dram_tensor("xm", [ROWS, 2 * D], BF16, kind="ExternalInput")
    out_acc = nc.dram_tensor("out_acc", [P, 6], F32, kind="ExternalOutput")
    out_sums = nc.dram_tensor("out_sums", [1, 1024], F32, kind="ExternalOutput")

    ctx = ExitStack()
    with ctx:
        ss = ctx.enter_context(nc.sbuf_tensor("ss", [P, NT * 2 * D], BF16))
        mm = ctx.enter_context(nc.sbuf_tensor("mm", [P, NT * 2 * D], BF16))
        a_q = ctx.enter_context(nc.sbuf_tensor("a_q", [P, 4 * D], BF16))
        e1_b = ctx.enter_context(nc.sbuf_tensor("e1_b", [P, 2 * D], BF16))
        d_b = ctx.enter_context(nc.sbuf_tensor("d_b", [P, D], BF16))
        u_b = ctx.enter_context(nc.sbuf_tensor("u_b", [P, D], BF16))
        m_b = ctx.enter_context(nc.sbuf_tensor("m_b", [P, 2 * D], BF16))
        ea_scr = ctx.enter_context(nc.sbuf_tensor("ea_scr", [P, 2 * D], BF16))
        ones = ctx.enter_context(nc.sbuf_tensor("ones", [P, 1], BF16))
        acc_e = ctx.enter_context(nc.sbuf_tensor("acc_e", [P, 6], F32))
        sums_sb = ctx.enter_context(nc.sbuf_tensor("sums_sb", [1, 1024], F32))
        psum_a = ctx.enter_context(nc.psum_tensor("psum_a", [1, 512], F32))
        psum_m = ctx.enter_context(nc.psum_tensor("psum_m", [1, 512], F32))

        ds = ctx.enter_context(nc.semaphore("ds"))      # xs tile arrivals (+16)
        dm = ctx.enter_context(nc.semaphore("dm"))      # xm tile arrivals (+16)
        v1 = ctx.enter_context(nc.semaphore("v1"))      # a(i) ready (+1/tile)
        v2 = ctx.enter_context(nc.semaphore("v2"))      # u(i) done: e3 slot free
        v3 = ctx.enter_context(nc.semaphore("v3"))      # m(i) ready
        a1 = ctx.enter_context(nc.semaphore("a1"))      # exp(a) instr done
        a2 = ctx.enter_context(nc.semaphore("a2"))      # e3(i) ready
        t_sem = ctx.enter_context(nc.semaphore("t_sem"))  # PE groups done
        o_sem = ctx.enter_context(nc.semaphore("o_sem"))  # ones ready
        c_sem = ctx.enter_context(nc.semaphore("c_sem"))  # psum copies done
        f_sem = ctx.enter_context(nc.semaphore("f_sem"))  # output DMAs done

        def sq(i):
            return ss[:, i * 2 * D : i * 2 * D + D]

        def sp(i):
            return ss[:, i * 2 * D + D : (i + 1) * 2 * D]

        def muq(i):
            return mm[:, i * 2 * D : i * 2 * D + D]

        def mup(i):
            return mm[:, i * 2 * D + D : (i + 1) * 2 * D]

        def a_slot(i):
            return a_q[:, (i % 4) * D : (i % 4 + 1) * D]

        def half(buf, i):
            return buf[:, (i % 2) * D : (i % 2 + 1) * D]

        with nc.Block(no_gpsimd_drain=True) as block:

            @block.sync
            def _(sync):
                for i in range(NT):
                    sync.dma_start(
                        ss[:, i * 2 * D : (i + 1) * 2 * D],
                        xs[i * P : (i + 1) * P, :],
                    ).then_inc(ds, 16)
                    sync.dma_start(
                        mm[:, i * 2 * D : (i + 1) * 2 * D],
                        xm[i * P : (i + 1) * P, :],
                    ).then_inc(dm, 16)
                sync.wait_ge(c_sem, 2)      # psum copies done (ready earlier)
                sync.dma_start(out_sums[:, :], sums_sb[:, :]).then_inc(f_sem, 16)
                sync.wait_ge(a1, 5)         # all acc_e exp cols written
                sync.wait_ge(v3, NT)        # tile-7 STT accum done
                sync.dma_start(out_acc[:, :], acc_e[:, :]).then_inc(f_sem, 16)
                sync.wait_ge(f_sem, 32)

            @block.vector
            def _(vector):
                vector.memset(ones[:, :], 1.0).then_inc(o_sem, 1)
                L = NT - 1
                H = D // 2
                for i in range(NT):
                    # a(i) = sigma_p - sigma_q
                    vector.wait_ge(ds, 16 * (i + 1))
                    if i >= 4:
                        # slot freed by its exp(a) instr and PE-a reader
                        vector.wait_ge(a1, {4: 1, 5: 1, 6: 2, 7: 2}[i])
                        vector.wait_ge(t_sem, 2 * (i - 4) + 1)
                    vector.tensor_sub(a_slot(i), sp(i), sq(i)).then_inc(v1, 1)
                    # d = mu_q - mu_p (single buffer: consumed by u next)
                    vector.wait_ge(dm, 16 * (i + 1))
                    vector.tensor_sub(d_b[:, :], muq(i), mup(i))
                    # u = d * e3
                    vector.wait_ge(a2, i + 1)
                    vector.tensor_mul(
                        u_b[:, :], d_b[:, :], half(e1_b, i)
                    ).then_inc(v2, 1)
                    if i >= 2:
                        vector.wait_ge(t_sem, 2 * (i - 2) + 2)  # PE-m(i-2) done
                    if i < L:
                        # m = u * u, reduced via PE ones-matmul
                        vector.tensor_mul(
                            half(m_b, i), u_b[:, :], u_b[:, :]
                        ).then_inc(v3, 1)
                    else:
                        # last tile: fused square+sum straight to the SBUF
                        # accumulator — keeps PE + psum-copy off the tail
                        vector.scalar_tensor_tensor(
                            half(m_b, i),
                            u_b[:, :],
                            1.0,
                            u_b[:, :],
                            Alu.mult,
                            Alu.mult,
                            accum_out=acc_e[:, 5:6],
                        ).then_inc(v3, 1)

            @block.scalar
            def _(scalar):
                # warmup: trigger the ~1.3us exp table load while DMA ramps
                scalar.activation(ea_scr[:, 0:1], ones[:, 0:1], Exp)
                def e3(i):
                    scalar.wait_ge(ds, 16 * (i + 1))
                    if i >= 2:
                        scalar.wait_ge(v2, i - 1)   # e3 slot freed by u(i-2)
                    scalar.activation(
                        half(e1_b, i), sq(i), Exp, scale=-0.5
                    ).then_inc(a2, 1)

                def expa(tiles, col):
                    scalar.wait_ge(v1, tiles[-1] + 1)   # a up to last tile ready
                    lo = tiles[0] % 4
                    w = len(tiles)
                    scalar.activation(
                        ea_scr[:, : w * D],
                        a_q[:, lo * D : (lo + w) * D],
                        Exp,
                        accum_out=acc_e[:, col : col + 1],
                    ).then_inc(a1, 1)

                for j in range(3):
                    e3(2 * j)
                    e3(2 * j + 1)
                    expa([2 * j, 2 * j + 1], j)
                e3(6)
                expa([6], 3)   # fits in the arrival gap before xs(7) lands
                e3(7)
                expa([7], 4)
                scalar.wait_ge(t_sem, 2 * NT - 2)   # PE-m(6): group-M fully done
                scalar.copy(sums_sb[0:1, 512:1024], psum_m[:, :]).then_inc(c_sem, 1)
                scalar.wait_ge(t_sem, 2 * NT - 1)   # PE-a(7): group-A fully done
                scalar.copy(sums_sb[0:1, 0:512], psum_a[:, :]).then_inc(c_sem, 1)

            @block.tensor
            def _(tensor):
                tensor.wait_ge(o_sem, 1)
                for i in range(NT):
                    tensor.wait_ge(v1, i + 1)
                    for c in range(4):
                        mmu = tensor.matmul(
                            psum_a[:, :],
                            ones[:, :],
                            a_slot(i)[:, 512 * c : 512 * (c + 1)],
                            start=(i == 0 and c == 0),
                            stop=(i == NT - 1 and c == 3),
                            skip_group_check=True,
                        )
                        if c == 3:
                            mmu.then_inc(t_sem, 1)
                    # tile NT-1's m is summed by the DVE STT, not PE
                    if i == NT - 1:
                        continue
                    tensor.wait_ge(v3, i + 1)
                    for c in range(4):
                        mmu = tensor.matmul(
                            psum_m[:, :],
                            ones[:, :],
                            half(m_b, i)[:, 512 * c : 512 * (c + 1)],
                            start=(i == 0 and c == 0),
                            stop=(i == NT - 2 and c == 3),
                            skip_group_check=True,
                        )
                        if c == 3:
                            mmu.then_inc(t_sem, 1)

    return nc


_NC = None


def _get_nc():
    global _NC
    if _NC is None:
        _NC = _build_nc()
    return _NC


def _pack(inputs):
    sq = np.asarray(inputs["sigma_q"], dtype=np.float32)
    sp = np.asarray(inputs["sigma_p"], dtype=np.float32)
    mq = np.asarray(inputs["mu_q"], dtype=np.float32)
    mp = np.asarray(inputs["mu_p"], dtype=np.float32)
    xs_full = np.stack([sq, sp], axis=1).reshape(B, 2 * D).astype(NPBF16)
    xm_full = np.stack([mq, mp], axis=1).reshape(B, 2 * D).astype(NPBF16)
    in_maps = [
        {
            "xs": np.ascontiguousarray(xs_full[c * ROWS : (c + 1) * ROWS]),
            "xm": np.ascontiguousarray(xm_full[c * ROWS : (c + 1) * ROWS]),
        }
        for c in range(NCORES)
    ]
    return in_maps


def _run(inputs, **kw):
    return run_bass_kernel_spmd(
        _get_nc(), _pack(inputs), core_ids=list(range(NCORES)), **kw
    )


def _combine(results):
    s_e = 0.0
    s_a = 0.0
    s_m = 0.0
    for r in results:
        acc = r["out_acc"].astype(np.float64)
        s_e += acc[:, 0:5].sum()
        s_m += acc[:, 5].sum()        # tile-7b Sum(u^2) via DVE STT
        sums = r["out_sums"].astype(np.float64)
        s_a += sums[0, 0:512].sum()   # = sum(sigma_p - sigma_q)
        s_m += sums[0, 512:1024].sum()
    kl = 0.5 * (-s_a + s_e + s_m - B * D)
    return np.asarray(kl, dtype=np.float32)


def kernel(**inputs):
    return _combine(_run(inputs).results)


def run_traced(inputs, **kw):
    br = _run(inputs, trace=True, **kw)
    return _combine(br.results), br
